# revision 1
# baseline (speedup 1.0000x reference)
"""Trainium2 Bass kernel for 8-head MHA (B=2, S=2048, d_model=512).

Sharding: core c -> batch b = c//4, head-pair hp = c%4 (heads 2hp, 2hp+1).
Each core computes q/k/v projections for its 128 out-dims (2 heads), the
masked-softmax attention for those heads, and the output-projection partial
for its 128 in-dims (heads summed on device). Host sums the 4 partials per
batch and adds the output bias.

On-chip layout is fully "transposed" (feature dims on partitions, sequence
on the free dim) so no activation transposes are ever needed.

Pipeline structure: four sub-phases (h0,qh0) (h1,qh0) (h0,qh1) (h1,qh1), kc
inner. Per kc step: two row-packed score matmuls (the head's dk=64 slice
duplicated into both partition halves so tile_position row-groups 0/64 run
concurrently), one exp ACT [128,1024], one DVE mask multiply, two PV
accumulation matmuls. ScalarE exp is the pacing engine; scores double-buffer
in PSUM (4 banks) + one xaug accumulator (2 banks) + misc pool (2 banks).
Normalization is emitted one sub-phase late so its PSUM broadcast never
blocks the next sub-phase's scores in the static TensorE queue. DMA issue is
spread over three descriptor queues: sync (x/w loads), scalar (v-path
loads), gpsimd/SWDGE (mask column-halves, output stores).
"""

import os
import sys
import types
import numpy as np
import ml_dtypes

HEAD = 8
D = 512
DK = 64
B = 2
N_CORES = 8
P = 128

_NC_CACHE = {}
LAST_RESULTS = None  # test harness reads BassKernelResults from here


def _register_ntff_hook():
    """Make run_bass_kernel_spmd(trace=True) work under axon by registering
    the NTFF profile hook that the trimmed antenv package lacks."""
    if "antenv.axon_hooks" in sys.modules:
        return
    try:
        import antenv

        mod = types.ModuleType("antenv.axon_hooks")
        _hook = [None]
        mod.set_axon_ntff_profile_hook = lambda h: _hook.__setitem__(0, h)
        mod.get_axon_ntff_profile_hook = lambda: _hook[0]
        sys.modules["antenv.axon_hooks"] = mod
        antenv.axon_hooks = mod
        if "/root/.axon_site" not in sys.path:
            sys.path.insert(0, "/root/.axon_site")
        from trn_agent_boot.trn_boot import _ntff_profile_via_ctypes

        mod.set_axon_ntff_profile_hook(
            _ntff_profile_via_ctypes("/opt/axon/libaxon_pjrt.so")
        )
    except Exception:
        pass  # tracing degrades; execution still works


def _build_nc(S):
    import concourse.tile as tile
    import concourse.mybir as mybir
    from concourse import bacc
    from concourse.bass import ts
    from contextlib import ExitStack

    f32 = mybir.dt.float32
    bf16 = mybir.dt.bfloat16
    AF = mybir.ActivationFunctionType
    debug = os.environ.get("MHA_DEBUG", "0") == "1"

    KC = S // P       # k chunks (score-tile rows == v s-blocks)
    EC = D // P       # embed chunks for projections
    HW = S // 2       # q half width (1024)

    nc = bacc.Bacc("TRN2", target_bir_lowering=False, debug=False,
                   num_devices=N_CORES)

    xqT = nc.dram_tensor("xqT", [D, S], bf16, kind="ExternalInput").ap()
    xkT = nc.dram_tensor("xkT", [D, S], bf16, kind="ExternalInput").ap()
    xvT = nc.dram_tensor("xvT", [D, S], bf16, kind="ExternalInput").ap()
    maskT = nc.dram_tensor("maskT", [S, S], bf16, kind="ExternalInput").ap()
    wqT = nc.dram_tensor("wqT", [D, P], bf16, kind="ExternalInput").ap()
    wkT = nc.dram_tensor("wkT", [D, P], bf16, kind="ExternalInput").ap()
    wvT = nc.dram_tensor("wvT", [D, P], bf16, kind="ExternalInput").ap()
    woT = nc.dram_tensor("woT", [P, D], bf16, kind="ExternalInput").ap()
    bq8 = nc.dram_tensor("bq8", [P, 1], f32, kind="ExternalInput").ap()
    bks = nc.dram_tensor("bks", [P, 1], f32, kind="ExternalInput").ap()
    bv_row = nc.dram_tensor("bv_row", [1, P], bf16, kind="ExternalInput").ap()
    outp = nc.dram_tensor("outp", [S, D], bf16, kind="ExternalOutput").ap()
    if debug:
        dbg_qd0 = nc.dram_tensor("dbg_qd0", [P, S], bf16, kind="ExternalOutput").ap()
        dbg_kd0 = nc.dram_tensor("dbg_kd0", [P, S], bf16, kind="ExternalOutput").ap()
        dbg_sc0 = nc.dram_tensor("dbg_sc0", [P, S // 2], f32, kind="ExternalOutput").ap()
        dbg_pt0 = nc.dram_tensor("dbg_pt0", [P, S // 2], bf16, kind="ExternalOutput").ap()
        dbg_xt00 = nc.dram_tensor("dbg_xt00", [DK + 1, S // 2], f32, kind="ExternalOutput").ap()
        dbg_xhat = nc.dram_tensor("dbg_xhat", [P, S], bf16, kind="ExternalOutput").ap()
        dbg_v = nc.dram_tensor("dbg_v", [P, (S // P) * 130], bf16, kind="ExternalOutput").ap()

    VG = 130  # v_sb column group: [v_h0(64) | 1 | v_h1(64) | 1]

    with tile.TileContext(nc) as tc, ExitStack() as ctx:
        consts = ctx.enter_context(tc.tile_pool(name="consts", bufs=1))
        resid = ctx.enter_context(tc.tile_pool(name="resid", bufs=1))
        mpool = ctx.enter_context(tc.tile_pool(name="maskp", bufs=KC))
        ppool = ctx.enter_context(tc.tile_pool(name="pp", bufs=4))
        xtpool = ctx.enter_context(tc.tile_pool(name="xtp", bufs=3))
        opool = ctx.enter_context(tc.tile_pool(name="outsb", bufs=2))
        if debug:
            dbgpool = ctx.enter_context(tc.tile_pool(name="dbgp", bufs=1))

        # ---- constants / weights (sync queue: q/k path; scalar: v path) ----
        ones_row = consts.tile([1, P], bf16)
        nc.vector.memset(ones_row[:], 1.0)
        ones1f = consts.tile([P, DK], f32)  # row DK used as bcast stationary
        nc.vector.memset(ones1f[:], 1.0)

        wq_sb = consts.tile([P, D], bf16)  # [p=e%128, ec*128+dk]
        wk_sb = consts.tile([P, D], bf16)
        wv_sb = consts.tile([P, D], bf16)
        wo_sb = consts.tile([P, D], bf16)
        bq8_sb = consts.tile([P, 1], f32)
        bks_sb = consts.tile([P, 1], f32)
        bvr_sb = consts.tile([1, P], bf16)

        # residents: per-head duplicated q/k (head slice in BOTH partition
        # halves, enabling concurrent row-group-0/64 score matmuls)
        qd = [resid.tile([P, S], bf16, name=f"qd{h}") for h in range(2)]
        kd = [resid.tile([P, S], bf16, name=f"kd{h}") for h in range(2)]
        v_sb = resid.tile([P, KC * VG], bf16)
        nc.vector.memset(v_sb[:], 1.0)  # pre-set the ones columns
        xhat = resid.tile([P, S], bf16)

        mask_t = [mpool.tile([P, S], bf16, tag="mask", name=f"mask{kc}")
                  for kc in range(KC)]
        mgate = consts.tile([1, 16], bf16)  # SWDGE dep-gate scratch

        # ---- projections ----
        xvpool = ctx.enter_context(tc.tile_pool(name="xvp", bufs=EC))
        with tc.tile_pool(name="qk_ps", bufs=2, space="PSUM") as qk_ps, \
             tc.tile_pool(name="xs", bufs=6 if debug else 8) as xs_pool:

            def emit_x_load(pool, srcT, tag, eng):
                tiles = []
                for ec in range(EC):
                    t = pool.tile([P, S], bf16, tag=tag, name=f"{tag}{ec}")
                    eng.dma_start(t[:], srcT[ec * P:(ec + 1) * P, :])
                    tiles.append(t)
                return tiles

            def emit_proj(w_sb, x_t, dst, bias_sb, scale):
                # two [128,1024] psum tiles per projection so the eviction of
                # one half overlaps the matmuls of the other / next projection
                for half in range(2):
                    ps = qk_ps.tile([P, HW], f32, tag="qk")
                    for ec in range(EC):
                        for st in range(2):
                            nc.tensor.matmul(
                                ps[:, ts(st, 512)], w_sb[:, ts(ec, P)],
                                x_t[ec][:, ts(half * 2 + st, 512)],
                                start=(ec == 0), stop=(ec == EC - 1))
                    # evict into dup layout: h0 -> partitions 0:64 of dst[0],
                    # h1 -> partitions 64:128 of dst[1]
                    hs = slice(half * HW, (half + 1) * HW)
                    nc.scalar.activation(dst[0][0:DK, hs], ps[0:DK, :],
                                         AF.Identity, bias=bias_sb[0:DK],
                                         scale=scale)
                    nc.scalar.activation(dst[1][DK:P, hs], ps[DK:P, :],
                                         AF.Identity, bias=bias_sb[DK:P],
                                         scale=scale)
                # DMA fills the mirror partition half (scalar HWDGE queue --
                # the sync queue is still busy issuing the bulk loads)
                nc.scalar.dma_start(dst[0][DK:P, :], dst[0][0:DK, :])
                nc.scalar.dma_start(dst[1][0:DK, :], dst[1][DK:P, :])

            # sync queue, in urgency order; tiny bias DMAs first so their
            # completion-semaphore lanes never alias behind bulk transfers
            nc.sync.dma_start(bq8_sb[:], bq8[:])
            nc.sync.dma_start(bks_sb[:], bks[:])
            nc.sync.dma_start(bvr_sb[:], bv_row[:])
            xq_t = emit_x_load(xs_pool, xqT, "xq", nc.sync)
            for ec in range(EC):
                nc.sync.dma_start(wq_sb[:, ts(ec, P)], wqT[ec * P:(ec + 1) * P, :])
            xk_t = emit_x_load(xs_pool, xkT, "xk", nc.sync)
            for ec in range(EC):
                nc.sync.dma_start(wk_sb[:, ts(ec, P)], wkT[ec * P:(ec + 1) * P, :])
            xv_t = emit_x_load(xvpool, xvT, "xv", nc.sync)
            for ec in range(EC):
                nc.sync.dma_start(wv_sb[:, ts(ec, P)], wvT[ec * P:(ec + 1) * P, :])
            nc.sync.dma_start(wo_sb[:], woT[:])

            emit_proj(wq_sb, xq_t, qd, bq8_sb, 0.0625)
            emit_proj(wk_sb, xk_t, kd, bks_sb, 1.0)

            # mask loads ride the otherwise-idle SWDGE (gpsimd) queue, gated
            # behind the xk load so they don't hog HBM before the projection
            # inputs land; emitted last so no phase-0 consumer's completion
            # lane aliases behind them
            nc.gpsimd.tensor_copy(mgate[:], xk_t[EC - 1][0:1, 0:16])
            for kc in range(KC):
                nc.gpsimd.dma_start(mask_t[kc][:, 0:HW],
                                    maskT[kc * P:(kc + 1) * P, 0:HW])
            for kc in range(KC):
                nc.gpsimd.dma_start(mask_t[kc][:, HW:S],
                                    maskT[kc * P:(kc + 1) * P, HW:S])

        # ---- attention: flat two-stream pipeline with a 2-step skew ----
        # scores/exp/mask of step i are emitted together with PV of step
        # i-SKEW, so the next sub-phase's scores always precede the previous
        # sub-phase's last PV matmuls in the static TensorE order.
        scores_ps = ctx.enter_context(
            tc.tile_pool(name="sc_ps", bufs=2, space="PSUM"))
        xaug_ps = ctx.enter_context(
            tc.tile_pool(name="xa_ps", bufs=1, space="PSUM"))
        misc_ps = ctx.enter_context(
            tc.tile_pool(name="mi_ps", bufs=2, space="PSUM"))

        def vproj_unit(sb):
            def emit():
                vpt = misc_ps.tile([P, 512], f32, tag="mi", name=f"vp{sb}")
                for ec in range(EC):
                    nc.tensor.matmul(vpt[:, 0:P], xv_t[ec][:, ts(sb, P)],
                                     wv_sb[:, ts(ec, P)],
                                     start=(ec == 0), stop=False)
                nc.tensor.matmul(vpt[:, 0:P], ones_row[:], bvr_sb[:],
                                 start=False, stop=True)
                nc.vector.tensor_copy(v_sb[:, sb * VG: sb * VG + DK],
                                      vpt[:, 0:DK])
                nc.vector.tensor_copy(
                    v_sb[:, sb * VG + DK + 1: sb * VG + 2 * DK + 1],
                    vpt[:, DK:2 * DK])
            return emit

        def oproj_unit(qb, tail=False):
            def emit():
                op = misc_ps.tile([P, 512], f32, tag="mi")
                nc.tensor.matmul(op[:], xhat[:, ts(qb, P)], wo_sb[:],
                                 start=True, stop=True)
                ob = opool.tile([P, D], bf16, tag="ob")
                if tail:  # ScalarE is idle after the last exp
                    nc.scalar.copy(ob[:], op[:])
                else:
                    nc.vector.tensor_copy(ob[:], op[:])
                nc.sync.dma_start(outp[qb * P:(qb + 1) * P, :], ob[:])
            return emit

        SPs = [(0, 0), (1, 0), (0, 1), (1, 1)]
        steps = [(spi, h, qh, kc)
                 for spi, (h, qh) in enumerate(SPs) for kc in range(KC)]
        SKEW = 2
        # deferred work consumed one unit per scores-step: vproj inside sp0,
        # each sub-phase's norm inside the following one, first o-proj batch
        # inside sp2
        extras = {0: [vproj_unit(sb) for sb in range(KC)],
                  1: [], 2: [], 3: []}
        pts = {}
        xaugs = {}

        def norm_units(h, qh, xt):
            q0 = qh * HW
            units = []
            for q2 in range(2):
                def emit(q2=q2):
                    # denominator row broadcast via matmul, reciprocal in
                    # place (partition-aligned), then normalize into xhat
                    bc = misc_ps.tile([P, 512], f32, tag="mi")
                    nc.tensor.matmul(bc[0:DK, :], ones1f[DK:DK + 1, :],
                                     xt[DK:DK + 1, ts(q2, 512)],
                                     start=True, stop=True)
                    nc.vector.reciprocal_approx_fast(out=bc[0:DK, :],
                                                     in_=bc[0:DK, :])
                    nc.vector.tensor_mul(
                        xhat[h * DK:(h + 1) * DK,
                             q0 + q2 * 512: q0 + (q2 + 1) * 512],
                        xt[0:DK, ts(q2, 512)], bc[0:DK, :])
                units.append(emit)
            return units

        def sc_stream(i):
            spi, h, qh, kc = steps[i]
            q0 = qh * HW
            if kc == 0:
                xaugs[spi] = xaug_ps.tile([DK + 1, HW], f32, tag="xaug",
                                          name=f"xa{spi}")
            ex = extras[spi]
            if ex and (kc >= 2 or spi == 0):
                ex.pop(0)()
            sc = scores_ps.tile([P, HW], f32, tag="sc")
            # K=128 via the duplicated operands: contraction sums the head
            # twice (q is pre-scaled by 1/16 to cancel it); the full
            # 128-partition stationary enables fast weight load
            nc.tensor.matmul(sc[:, 0:512], kd[h][:, ts(kc, P)],
                             qd[h][:, q0:q0 + 512], start=True, stop=True)
            nc.tensor.matmul(sc[:, 512:HW], kd[h][:, ts(kc, P)],
                             qd[h][:, q0 + 512:q0 + HW], start=True, stop=True)
            pt = ppool.tile([P, HW], bf16, tag="p")
            if debug and spi == 0 and kc == 0:
                dsc = dbgpool.tile([P, HW], f32, tag="dsc", name="dsc")
                nc.vector.tensor_copy(dsc[:], sc[:])
                nc.sync.dma_start(dbg_sc0[:], dsc[:])
            nc.scalar.activation(pt[:], sc[:], AF.Exp)
            nc.vector.tensor_mul(pt[:], pt[:], mask_t[kc][:, q0:q0 + HW])
            if debug and spi == 0 and kc == 0:
                nc.sync.dma_start(dbg_pt0[:], pt[:])
            pts[i] = pt

        def pv_stream(j):
            spi, h, qh, kc = steps[j]
            bv0 = (DK + 1) * h
            pt = pts.pop(j)
            for q2 in range(2):
                nc.tensor.matmul(
                    xaugs[spi][:, ts(q2, 512)],
                    v_sb[:, kc * VG + bv0: kc * VG + bv0 + DK + 1],
                    pt[:, ts(q2, 512)],
                    start=(kc == 0), stop=(kc == KC - 1))
            if kc == KC - 1:
                # prompt eviction frees the xaug banks; norm work is pushed
                # into the NEXT sub-phase's scores-stream
                xt = xtpool.tile([DK + 1, HW], f32, tag="xt")
                nc.vector.tensor_copy(xt[:], xaugs[spi][:])
                if debug and spi == 0:
                    nc.sync.dma_start(dbg_xt00[:], xt[:])
                if spi < 3:
                    extras[spi + 1].extend(norm_units(h, qh, xt))
                    if spi == 1:
                        extras[2].extend(oproj_unit(qb) for qb in range(8))
                else:
                    tail_norm.extend(norm_units(h, qh, xt))

        tail_norm = []
        for i in range(len(steps) + SKEW):
            if i < len(steps):
                sc_stream(i)
            if i >= SKEW:
                pv_stream(i - SKEW)

        # tail: interleave the last norm with the second o-proj batch
        tail_norm[0]()
        for qb in (8, 9, 10, 11):
            oproj_unit(qb, tail=True)()
        tail_norm[1]()
        for qb in (12, 13, 14, 15):
            oproj_unit(qb, tail=True)()

        if debug:
            nc.sync.dma_start(dbg_qd0[:], qd[0][:])
            nc.sync.dma_start(dbg_kd0[:], kd[0][:])
            nc.sync.dma_start(dbg_xhat[:], xhat[:])
            nc.sync.dma_start(dbg_v[:], v_sb[:])

    nc.compile()
    return nc


def _get_nc(S):
    if S not in _NC_CACHE:
        _NC_CACHE[S] = _build_nc(S)
    return _NC_CACHE[S]


def kernel(query, key, value, mask, Wq, bq, Wk, bk, Wv, bv, Wo, bo):
    global LAST_RESULTS
    trace = os.environ.get("MHA_TRACE", "0") == "1"
    if trace:
        _register_ntff_hook()

    from concourse.bass_utils import run_bass_kernel_spmd

    query = np.asarray(query)
    key = np.asarray(key)
    value = np.asarray(value)
    mask = np.asarray(mask)
    Wq, bq, Wk, bk = map(np.asarray, (Wq, bq, Wk, bk))
    Wv, bv, Wo, bo = map(np.asarray, (Wv, bv, Wo, bo))

    S = query.shape[1]
    nc = _get_nc(S)

    bf = ml_dtypes.bfloat16
    maskTb = np.ascontiguousarray((mask[0] != 0).T).astype(bf)
    xT = {}
    for b in range(B):
        xT[("q", b)] = np.ascontiguousarray(query[b].T).astype(bf)
        xT[("k", b)] = np.ascontiguousarray(key[b].T).astype(bf)
        xT[("v", b)] = np.ascontiguousarray(value[b].T).astype(bf)

    in_maps = []
    for c in range(N_CORES):
        b, hp = divmod(c, 4)
        sl = slice(P * hp, P * hp + P)
        in_maps.append({
            "xqT": xT[("q", b)],
            "xkT": xT[("k", b)],
            "xvT": xT[("v", b)],
            "maskT": maskTb,
            "wqT": np.ascontiguousarray(Wq[sl, :].T).astype(bf),
            "wkT": np.ascontiguousarray(Wk[sl, :].T).astype(bf),
            "wvT": np.ascontiguousarray(Wv[sl, :].T).astype(bf),
            "woT": np.ascontiguousarray(Wo[:, sl].T).astype(bf),
            "bq8": (bq[sl] / 16.0).reshape(P, 1).astype(np.float32),
            "bks": bk[sl].reshape(P, 1).astype(np.float32),
            "bv_row": bv[sl].reshape(1, P).astype(bf),
        })

    res = run_bass_kernel_spmd(
        nc, in_maps, core_ids=list(range(N_CORES)),
        trace=trace, trace_cores=[0] if trace else None,
    )
    LAST_RESULTS = res

    out = np.zeros((B, S, D), np.float32)
    for c in range(N_CORES):
        out[c // 4] += res.results[c]["outp"].astype(np.float32)
    out += bo.astype(np.float32)
    return out

